# revision 50
# baseline (speedup 1.0000x reference)
"""Linformer multi-head attention on 8 Trainium2 NeuronCores.

Sharding: data-parallel over batch (BATCH=8 -> 1 batch element per core).
Each core runs the full per-batch computation:
  q = x@wq, k = x@wk, v = x@wv            (per head h: 64-dim slices)
  k_proj[h] = E[h].T @ k[h]   [256, 64]   (contraction over seq)
  v_proj[h] = F[h].T @ v[h]   [256, 64]
  scores = q @ k_proj.T / 8   [4096, 256]
  attn = softmax(scores)  ;  out = attn @ v_proj
  y = concat_heads(out) @ w_out + b_out

Kernel layout strategy (per core):
  - x/E/F are pre-laid-out on the host (xT pre-transposed, E/F reshaped to
    [128, j, s, h, r]) so every DMA moves long contiguous per-partition runs.
  - Q is produced directly in transposed layout QT [512, 4096].
  - K/V are produced in natural layout per 128-row tile; the per-head
    k_projT/v_projT [64, 256] accumulate across the whole sequence directly
    in persistent PSUM banks (zero-seeded so start stays False).
  - scores are computed transposed (scoresT [256, ntile]) so softmax's exp is
    elementwise and the r-sum (denominator) comes free from the PV matmul via
    an appended ones-column on v_proj.
  - phase C is software-pipelined one half-group (4 heads) deep: score
    matmuls+exps of group n interleave with PV matmuls of group n-1, and the
    output dense of tile j-1 fills the reciprocal/broadcast window.
  - normalization: denominator rows gathered at partitions {0,32,64,96},
    one approx-reciprocal per 4 heads, broadcast via a one-hot sel matmul,
    one elementwise mul per head.
  - final dense y = outT.T @ w_out + b_out, stored bf16 (upcast on host).

Compute dtype is bf16 (inputs cast on host) with fp32 PSUM accumulation.
Set LINF_COMPUTE=f32 for a full-fp32 fallback.
"""

import os

import numpy as np
import ml_dtypes

BATCH, SEQ, DM = 8, 4096, 512
NH, DH, R = 8, 64, 256
NCORES = 8
NT = SEQ // 512  # 8 big n-tiles of 512 rows
COMPUTE = os.environ.get("LINF_COMPUTE", "bf16")

_built = {}


def _build():
    """Build the Bass module (once per process)."""
    if "nc" in _built:
        return _built["nc"]

    from contextlib import ExitStack

    import concourse.bass as bass
    import concourse.bacc as bacc
    import concourse.mybir as mybir
    import concourse.tile as tile
    from concourse.masks import make_identity

    f32 = mybir.dt.float32
    cdt = mybir.dt.bfloat16 if COMPUTE == "bf16" else f32

    nc = bacc.Bacc("TRN2", target_bir_lowering=False, debug=False)

    # host-prelayouted tensors (see kernel()): DMA-friendly, long
    # contiguous per-partition runs, xT already transposed.
    # x:   [128, NT, 4, 512]   x[p, j, dk, n] = x_orig[j*512+n, dk*128+p]
    # E/F: [128, NT, 4, NH, R] E[p, j, s, h, r] = E_orig[h, j*512+s*128+p, r]
    x_d = nc.dram_tensor("x", [128, NT, 4, 512], cdt, kind="ExternalInput").ap()
    wq_d = nc.dram_tensor("wq", [DM, DM], cdt, kind="ExternalInput").ap()
    wk_d = nc.dram_tensor("wk", [DM, DM], cdt, kind="ExternalInput").ap()
    wv_d = nc.dram_tensor("wv", [DM, DM], cdt, kind="ExternalInput").ap()
    e_d = nc.dram_tensor("E", [128, NT, 4, NH, R], cdt, kind="ExternalInput").ap()
    f_d = nc.dram_tensor("F", [128, NT, 4, NH, R], cdt, kind="ExternalInput").ap()
    wo_d = nc.dram_tensor("w_out", [DM, DM], cdt, kind="ExternalInput").ap()
    b_d = nc.dram_tensor("b_out", [DM], f32, kind="ExternalInput").ap()
    # one-hot selector for the denominator broadcast (built host-side):
    # selc[p, hpin, c] = 1 iff p == 32*(2*hpin + c//64)
    sel_d = nc.dram_tensor("selc", [128, 2, 128], cdt, kind="ExternalInput").ap()
    y_d = nc.dram_tensor("y", [SEQ, DM], cdt, kind="ExternalOutput").ap()

    with tile.TileContext(nc) as tc, ExitStack() as ctx:
        singles = ctx.enter_context(tc.tile_pool(name="singles", bufs=1))

        # DMA emission order = HWDGE FIFO order: issue what the first
        # matmuls need (wq, x tile 0) before the phase-C-only constants.
        w_sb = {}
        for name, d in (("wq", wq_d), ("wk", wk_d), ("wv", wv_d), ("wo", wo_d)):
            w_sb[name] = singles.tile([128, 4, DM], cdt, name=f"w_{name}")
        nc.sync.dma_start(
            out=w_sb["wq"], in_=wq_d.rearrange("(dk p) m -> p dk m", p=128)
        )

        ident = singles.tile([128, 128], cdt)
        sel4 = singles.tile([128, 2, 128], cdt)
        den_g = [singles.tile([128, 512], f32, name=f"den{i}") for i in range(4)]
        bias_bc = singles.tile([128, DM], f32)

        def emit_wkv_dma():
            for name, d in (("wk", wk_d), ("wv", wv_d)):
                nc.sync.dma_start(
                    out=w_sb[name], in_=d.rearrange("(dk p) m -> p dk m", p=128)
                )

        def emit_singles_rest():
            """Constants only needed in phase C (emitted late so the first
            j-tile's loads go to the DMA rings first)."""
            nc.sync.dma_start(
                out=w_sb["wo"], in_=wo_d.rearrange("(dk p) m -> p dk m", p=128)
            )
            # sel4[hpin]: [128,128] one-hot lhsT; column c has a 1 at row
            # 32*(2*hpin + c//64) -> broadcast matmul expands each
            # denominator-reciprocal row across 64 output partitions.
            nc.sync.dma_start(out=sel4, in_=sel_d)
            b_bc_ap = bass.AP(
                tensor=b_d.tensor, offset=b_d.offset, ap=[[0, 128]] + list(b_d.ap)
            )
            nc.sync.dma_start(out=bias_bc, in_=b_bc_ap)
            make_identity(nc, ident)
            # denominator staging: head r of a 4-head group at partition 32r
            # (engine ops may only start at partitions 0/32/64/96); memset to
            # 1.0 so untouched rows stay finite through reciprocal.
            for t in den_g:
                nc.vector.memset(t, 1.0)

        # QT global [512, 4096] as 4 tiles [128, 4096]; tile t = heads 2t,2t+1
        qt_g = [singles.tile([128, SEQ], cdt, tag=f"qt{t}", name=f"qt{t}") for t in range(4)]
        # per-head low-rank projections, transposed [64, 256], packed 4/tile:
        # head h -> tile t=h//4, partition half ph=(h//2)%2... see hslice()
        kpT_sb = [singles.tile([128, 2 * R], cdt, tag=f"kp{t}", name=f"kpT{t}") for t in range(2)]
        vpT_sb = [singles.tile([128, 2 * R], cdt, tag=f"vp{t}", name=f"vpT{t}") for t in range(2)]

        def hslice(sb, h):
            """[64, 256] slice of packed kpT/vpT for head h."""
            t, ph, ch = h // 4, h % 2, (h // 2) % 2
            return sb[t][ph * 64 : (ph + 1) * 64, ch * R : (ch + 1) * R]

        # v_proj natural chunks + ones column: [128, 2, 65] per head
        vext = singles.tile([128, NH, 2, 65], cdt)

        # zero stationary for seeding PSUM accumulator banks
        zero128 = singles.tile([128, 128], cdt)
        nc.vector.memset(zero128, 0.0)

        # ---------------- Phase AB: QT, k_projT, v_projT ----------------
        with (
            tc.tile_pool(name="p_xt", bufs=4) as p_xt,
            tc.tile_pool(name="p_ef", bufs=3) as p_ef,
            tc.tile_pool(name="p_kv", bufs=10) as p_kv,
            tc.tile_pool(name="ps_t", bufs=2, space="PSUM") as ps_t,
            tc.tile_pool(name="ps_mm", bufs=2, space="PSUM") as ps_mm,
            tc.tile_pool(name="ps_pp", bufs=4, space="PSUM") as ps_pp,
        ):
            # persistent PSUM accumulators for k_projT/v_projT: all proj
            # matmuls accumulate with start=False across the whole j loop.
            # Seed each bank with a zero matmul (sets has_written everywhere
            # so interleaved per-quadrant chains never re-clear the bank).
            pp = {
                kv: [
                    ps_pp.tile([128, 2 * R], f32, tag="pp", name=f"pp_{kv}_{t}")
                    for t in range(2)
                ]
                for kv in ("k", "v")
            }
            for kv in ("k", "v"):
                for t in range(2):
                    nc.tensor.matmul(
                        pp[kv][t],
                        zero128,
                        w_sb["wq"][:, 0, :],
                        start=True,
                        stop=False,
                        skip_group_check=True,
                    )

            for j in range(NT):  # 8 n-tiles of 512 rows
                # xT for this j comes straight from the host-pretransposed x
                xt_all = p_xt.tile([128, 4, 512], cdt, tag="xt", name=f"xt_{j}")
                nc.sync.dma_start(out=xt_all, in_=x_d[:, j, :, :])
                xT = [xt_all[:, dk, :] for dk in range(4)]

                # QT_j[dq] [128, 512] = sum_dk wq[dk,dq-chunk].T-form @ xT[dk]
                # dk-outer order: consecutive matmuls hit different PSUM
                # banks so fill/drain overlap instead of serializing.
                pqs = [ps_mm.tile([128, 512], f32, tag="pmm", name=f"pq_{j}_{i}") for i in range(4)]
                for dk in range(4):
                    for dq in range(4):
                        nc.tensor.matmul(
                            pqs[dq],
                            w_sb["wq"][:, dk, dq * 128 : (dq + 1) * 128],
                            xT[dk],
                            start=(dk == 0),
                            stop=(dk == 3),
                        )
                for dq in range(4):
                    nc.vector.tensor_copy(
                        qt_g[dq][:, j * 512 : (j + 1) * 512], pqs[dq]
                    )

                if j == 0:
                    emit_wkv_dma()
                # E/F tiles + K/V for the 4 subtiles of this j (one DMA each)
                e_all = p_ef.tile([128, 4, NH, R], cdt, tag="ef_e", name=f"e_{j}")
                f_all = p_ef.tile([128, 4, NH, R], cdt, tag="ef_f", name=f"f_{j}")
                nc.sync.dma_start(out=e_all, in_=e_d[:, j, :, :, :])
                nc.sync.dma_start(out=f_all, in_=f_d[:, j, :, :, :])
                if j == 0:
                    emit_singles_rest()
                e_ts = [e_all[:, s, :, :] for s in range(4)]
                f_ts = [f_all[:, s, :, :] for s in range(4)]
                k_sbs, v_sbs = [], []
                for s in range(4):
                    # k/v interleaved: alternating PSUM banks + shared
                    # stationary xT slice between the adjacent matmuls
                    pks = {
                        w: ps_mm.tile([128, 512], f32, tag="pmm", name=f"pk_{j}_{s}_{w}")
                        for w in ("wk", "wv")
                    }
                    for dk in range(4):
                        for wname in ("wk", "wv"):
                            nc.tensor.matmul(
                                pks[wname],
                                xT[dk][:, s * 128 : (s + 1) * 128],
                                w_sb[wname][:, dk, :],
                                start=(dk == 0),
                                stop=(dk == 3),
                            )
                    for wname, dest in (("wk", k_sbs), ("wv", v_sbs)):
                        kv_sb = p_kv.tile([128, 512], cdt, tag="kv", name=f"kv_{j}_{s}_{wname}")
                        nc.vector.tensor_copy(kv_sb, pks[wname])
                        dest.append(kv_sb)

                # k/v projection partials accumulate straight into the
                # persistent PSUM banks (seeded above, so start stays False)
                for kv_list, ef_list, kv in (
                    (k_sbs, e_ts, "k"),
                    (v_sbs, f_ts, "v"),
                ):
                    for t in range(2):
                        for hh in range(4):
                            h = t * 4 + hh
                            ph, chh = h % 2, (h // 2) % 2
                            for s in range(4):
                                nc.tensor.matmul(
                                    pp[kv][t][
                                        ph * 64 : (ph + 1) * 64,
                                        chh * R : (chh + 1) * R,
                                    ],
                                    kv_list[s][:, h * 64 : (h + 1) * 64],
                                    ef_list[s][:, h, :],
                                    start=False,
                                    stop=(j == NT - 1 and s == 3),
                                    skip_group_check=True,
                                )

            for t in range(2):
                nc.vector.tensor_copy(kpT_sb[t], pp["k"][t])
                nc.vector.tensor_copy(vpT_sb[t], pp["v"][t])

            # build vext: transpose v_projT[h] chunks to natural + ones col
            for h in range(NH):
                pv = ps_t.tile([128, 128], cdt, tag="pt", name="pv")
                for rc in range(2):
                    nc.tensor.transpose(
                        pv[:, rc * 64 : (rc + 1) * 64],
                        hslice(vpT_sb, h)[:, rc * 128 : (rc + 1) * 128],
                        ident[(h % 2) * 64 : (h % 2) * 64 + 64, (h % 2) * 64 : (h % 2) * 64 + 64],
                    )
                for rc in range(2):
                    nc.vector.tensor_copy(
                        vext[:, h, rc, 0:64], pv[:, rc * 64 : (rc + 1) * 64]
                    )
                nc.vector.memset(vext[:, h, :, 64:65], 1.0)

        # ---------------- Phase C: attention + output dense ----------------
        y_r = y_d.rearrange("(j s p) m -> p j s m", s=4, p=128)
        with (
            tc.tile_pool(name="p_at", bufs=16) as p_at,
            tc.tile_pool(name="p_den", bufs=3) as p_den,
            tc.tile_pool(name="p_rec", bufs=2) as p_rec,
            tc.tile_pool(name="p_ot", bufs=8) as p_ot,
            tc.tile_pool(name="p_fin", bufs=2) as p_fin,
            tc.tile_pool(name="ps_sc", bufs=3, space="PSUM") as ps_sc,
            tc.tile_pool(name="ps_out", bufs=4, space="PSUM") as ps_out,
            tc.tile_pool(name="ps_bc", bufs=1, space="PSUM") as ps_bc,
        ):
            def emit_fin(j, oT):
                """Output dense for tile j (lagged behind attention)."""
                fin = p_fin.tile([128, 4, 512], cdt, tag="fin", name=f"fin_{j}")
                for sp in range(2):
                    # two interleaved s-chains -> alternating PSUM banks
                    fps = [ps_sc.tile([128, 512], f32, tag="sc", name=f"fp_{j}_{sp}_{i}") for i in range(2)]
                    for dm in range(4):
                        for si in range(2):
                            s = sp * 2 + si
                            nc.tensor.matmul(
                                fps[si],
                                oT[dm][:, s * 128 : (s + 1) * 128],
                                w_sb["wo"][:, dm, :],
                                start=(dm == 0),
                                stop=(dm == 3),
                            )
                    for si in range(2):
                        with nc.allow_low_precision(reason="bf16 output store"):
                            nc.vector.tensor_add(
                                fin[:, sp * 2 + si, :], fps[si], bias_bc
                            )
                nc.sync.dma_start(out=y_r[:, j, :, :], in_=fin)

            # score step order: head pairs adjacent (h even uses array rows
            # 0-63, h odd rows 64-127) so paired matmuls run concurrently in
            # disjoint row groups.
            SC_ORDER = [(0, 0), (1, 0), (0, 1), (1, 1), (2, 0), (3, 0), (2, 1), (3, 1)]

            def emit_scores(cur):
                """8 score matmuls + exps for a 4-head half-group; PV of the
                previous half-group interleaved step-by-step so the PE never
                stalls on ACT exp (in-order engine)."""
                j, half = cur["j"], cur["half"]
                cur["ats"] = {}
                for k, rc in SC_ORDER:
                    h = half * 4 + k
                    qrow = qt_g[h // 2][
                        (h % 2) * 64 : (h % 2) * 64 + 64,
                        j * 512 : (j + 1) * 512,
                    ]
                    sc = ps_sc.tile([128, 512], f32, tag="sc")
                    nc.tensor.matmul(
                        sc,
                        hslice(kpT_sb, h)[:, rc * 128 : (rc + 1) * 128],
                        qrow,
                        start=True,
                        stop=True,
                    )
                    a = p_at.tile([128, 512], cdt, tag="at")
                    nc.scalar.activation(
                        a, sc, mybir.ActivationFunctionType.Exp, scale=0.125
                    )
                    cur["ats"][(k, rc)] = a
                    yield  # interleave point: PV step of previous half-group

            def emit_pv_step(ph, i):
                """PV matmul step i (head i//2, rc i%2) of half-group ph."""
                k, rc = i // 2, i % 2
                h = ph["half"] * 4 + k
                if rc == 0:
                    ph["ops"].append(ps_out.tile([128, 512], f32, tag="op", name=f"op_{ph['j']}_{ph['half']}_{k}"))
                op = ph["ops"][k]
                nc.tensor.matmul(
                    op[0:65, :],
                    vext[:, h, rc, :],
                    ph["ats"][(k, rc)],
                    start=(rc == 0),
                    stop=(rc == 1),
                )
                if rc == 1:
                    # denominator row -> staging tile (DVE only: putting these
                    # PSUM-dependent copies on ACT head-of-line blocks the
                    # strict-FIFO exp stream)
                    nc.vector.tensor_copy(
                        ph["den"][32 * k : 32 * k + 1, :], op[64:65, :]
                    )

            def emit_norm(ph):
                """Reciprocal (batched over 4 heads) + bf16 cast."""
                recf = p_rec.tile([128, 512], f32, tag="recf")
                nc.vector.reciprocal_approx_fast(out=recf, in_=ph["den"])
                recb = p_rec.tile([128, 512], cdt, tag="recb")
                nc.vector.tensor_copy(recb, recf)
                ph["recb"] = recb

            def emit_mul(ph):
                for hpin in range(2):
                    hp = ph["half"] * 2 + hpin
                    # bc[hh*64+u, :] = recb[32*(2*hpin+hh), :] via sel4
                    bc = ps_bc.tile([128, 512], f32, tag="bc", name=f"bc_{ph['j']}_{ph['half']}_{hpin}")
                    nc.tensor.matmul(
                        bc, sel4[:, hpin, :], ph["recb"], start=True, stop=True
                    )
                    bc_sb = p_den.tile([128, 512], f32, tag="bcs")
                    nc.scalar.copy(bc_sb, bc)
                    for hh in range(2):
                        k = hpin * 2 + hh
                        nc.vector.tensor_mul(
                            ph["oT"][hp][hh * 64 : (hh + 1) * 64, :],
                            ph["ops"][k][0:64, :],
                            bc_sb[hh * 64 : (hh + 1) * 64, :],
                        )

            prevh = None
            fin_q = []
            oT = None
            for n in range(2 * NT):
                j, half = n // 2, n % 2
                if half == 0:
                    oT = [
                        p_ot.tile([128, 512], cdt, tag="ot", name=f"oT{j}_{t}")
                        for t in range(4)
                    ]
                cur = {
                    "j": j, "half": half, "den": den_g[n % 4],
                    "ats": [], "ops": [], "oT": oT,
                }
                gen = emit_scores(cur)
                for i, _ in enumerate(gen):
                    if prevh is not None:
                        emit_pv_step(prevh, i)
                if prevh is not None:
                    emit_norm(prevh)
                if fin_q:
                    emit_fin(*fin_q.pop(0))  # fills the reciprocal window
                if prevh is not None:
                    emit_mul(prevh)
                    if prevh["half"] == 1:
                        fin_q.append((prevh["j"], prevh["oT"]))
                prevh = cur
            # epilogue: drain the last half-group + remaining output denses
            for i in range(8):
                emit_pv_step(prevh, i)
            emit_norm(prevh)
            if fin_q:
                emit_fin(*fin_q.pop(0))
            emit_mul(prevh)
            fin_q.append((prevh["j"], prevh["oT"]))
            while fin_q:
                emit_fin(*fin_q.pop(0))

    nc.compile()
    _built["nc"] = nc
    return nc


def _runner():
    """Build (once) a cached jitted 8-core executor for the Bass module."""
    if "run" in _built:
        return _built["run"]

    import jax
    import numpy as _np

    import concourse.mybir as mybir
    from concourse import bass2jax

    bass2jax.install_neuronx_cc_hook()
    nc = _build()

    part_name = nc.partition_id_tensor.name if nc.partition_id_tensor else None
    in_names, out_names, out_avals = [], [], []
    for alloc in nc.m.functions[0].allocations:
        if not isinstance(alloc, mybir.MemoryLocationSet):
            continue
        name = alloc.memorylocations[0].name
        if alloc.kind == "ExternalInput":
            if name != part_name:
                in_names.append(name)
        elif alloc.kind == "ExternalOutput":
            out_names.append(name)
            out_avals.append(
                jax.core.ShapedArray(
                    tuple(alloc.tensor_shape), mybir.dt.np(alloc.dtype)
                )
            )
    n_params = len(in_names)
    n_outs = len(out_avals)
    all_in_names = tuple(
        in_names + out_names + ([part_name] if part_name else [])
    )

    import jax.numpy as jnp
    from jax.sharding import NamedSharding

    def _body(*args):
        operands = list(args)
        if part_name is not None:
            operands.append(bass2jax.partition_id_tensor())
        outs = bass2jax._bass_exec_p.bind(
            *operands,
            out_avals=tuple(out_avals),
            in_names=all_in_names,
            out_names=tuple(out_names),
            lowering_input_output_aliases=(),
            sim_require_finite=True,
            sim_require_nnan=True,
            nc=nc,
        )
        return tuple(outs)

    devices = jax.devices()[:NCORES]
    mesh = bass2jax.Mesh(_np.asarray(devices), ("core",))
    p_core = bass2jax.PartitionSpec("core")
    p_repl = bass2jax.PartitionSpec()
    # "x" is per-core; every other input is replicated across cores.
    # zero output buffers ride along as per-core params (hook requires params).
    in_specs = tuple(p_core if n == "x" else p_repl for n in in_names) + (
        p_core,
    ) * n_outs
    sharded = jax.jit(
        bass2jax.shard_map(
            _body,
            mesh=mesh,
            in_specs=in_specs,
            out_specs=(p_core,) * n_outs,
            check_rep=False,
        ),
        keep_unused=True,
    )
    sh_core = NamedSharding(mesh, p_core)
    sh_repl = NamedSharding(mesh, p_repl)
    dev_cache = {}

    zero_cache = {}

    def run(in_maps):
        args = []
        for name in in_names:
            if name == "x":
                xc = np.concatenate([np.asarray(m[name]) for m in in_maps], axis=0)
                args.append(jax.device_put(xc, sh_core))
            else:
                a = np.asarray(in_maps[0][name])
                key = (name, a.shape, str(a.dtype), hash(a.tobytes()))
                if key not in dev_cache:
                    dev_cache.clear() if len(dev_cache) > 64 else None
                    dev_cache[key] = jax.device_put(a, sh_repl)
                args.append(dev_cache[key])
        for i, a in enumerate(out_avals):
            if i not in zero_cache:
                zero_cache[i] = jax.device_put(
                    np.zeros((NCORES * a.shape[0], *a.shape[1:]), a.dtype), sh_core
                )
            args.append(zero_cache[i])
        out_arrs = sharded(*args)
        return [
            {
                name: np.asarray(out_arrs[i]).reshape(
                    NCORES, *out_avals[i].shape
                )[c]
                for i, name in enumerate(out_names)
            }
            for c in range(NCORES)
        ]

    _built["run"] = run
    return run


def _prelayout_x(xi, np_c):
    """[4096, 512] -> [128, NT, 4, 512]: x_pre[p, j, dk, n] = x[j*512+n, dk*128+p]."""
    xt = np.ascontiguousarray(xi, dtype=np_c).T  # [512, 4096]
    xt = xt.reshape(4, 128, NT, 512).transpose(1, 2, 0, 3)
    return np.ascontiguousarray(xt)


def _prelayout_ef(e, np_c):
    """[NH, SEQ, R] -> [128, NT, 4, NH, R]: e_pre[p,j,s,h,r] = E[h, j*512+s*128+p, r]."""
    t = np.asarray(e, dtype=np_c).transpose(1, 0, 2)  # [SEQ, NH, R]
    t = t.reshape(NT, 4, 128, NH, R).transpose(2, 0, 1, 3, 4)
    return np.ascontiguousarray(t)


def kernel(x, wq, wk, wv, E, F, w_out, b_out):
    """Full inputs in, full output out. Shards batch across 8 cores."""
    run = _runner()

    np_c = ml_dtypes.bfloat16 if COMPUTE == "bf16" else np.float32
    wq_c = np.ascontiguousarray(wq, dtype=np_c)
    wk_c = np.ascontiguousarray(wk, dtype=np_c)
    wv_c = np.ascontiguousarray(wv, dtype=np_c)
    e_c = _prelayout_ef(E, np_c)
    f_c = _prelayout_ef(F, np_c)
    wo_c = np.ascontiguousarray(w_out, dtype=np_c)
    b_c = np.ascontiguousarray(b_out, dtype=np.float32)
    sel_c = np.zeros((128, 2, 128), dtype=np_c)
    for hpin in range(2):
        for hh in range(2):
            sel_c[32 * (2 * hpin + hh), hpin, hh * 64 : (hh + 1) * 64] = 1.0

    in_maps = [
        {
            "x": _prelayout_x(x[i], np_c),
            "wq": wq_c,
            "wk": wk_c,
            "wv": wv_c,
            "E": e_c,
            "F": f_c,
            "w_out": wo_c,
            "b_out": b_c,
            "selc": sel_c,
        }
        for i in range(NCORES)
    ]
    results = run(in_maps)
    return np.stack(
        [results[i]["y"] for i in range(NCORES)], axis=0
    ).astype(np.float32)


if __name__ == "__main__":
    xs = {
        "x": np.random.randn(BATCH, SEQ, DM).astype(np.float32),
        "wq": np.random.randn(DM, DM).astype(np.float32) * 0.05,
        "wk": np.random.randn(DM, DM).astype(np.float32) * 0.05,
        "wv": np.random.randn(DM, DM).astype(np.float32) * 0.05,
        "E": np.random.randn(NH, SEQ, R).astype(np.float32) * 0.03,
        "F": np.random.randn(NH, SEQ, R).astype(np.float32) * 0.03,
        "w_out": np.random.randn(DM, DM).astype(np.float32) * 0.05,
        "b_out": np.zeros(DM, np.float32),
    }
    y = kernel(**xs)
    print(y.shape, y.dtype)

